# revision 24
# baseline (speedup 1.0000x reference)
"""Controlled-rotation (CR) kernel for Trainium2, 8-core SPMD.

Math (D=2, control=0, target=1, j=0,k=1  =>  S = Pauli-X):
  x viewed as (control d, target t, rest r) with rest = N/4.
  d=0: identity.  d=1: apply RX(angle): M = cos(a/2) I - i sin(a/2) X.
  With c=cos(a/2), s=sin(a/2), u=x[1,0], v=x[1,1]:
    out_u = (c*ur + s*vi) + i(c*ui - s*vr)
    out_v = (c*vr + s*ui) + i(c*vi - s*ur)
  Output = stack([real, imag]) of shape (2, N), float32.

Sharding: rest axis split evenly across 8 cores (no communication).
Per core: d=0 half is a DRAM->DRAM DMA copy; d=1 half is loaded to SBUF,
rotated with DVE multiply-accumulate ops, stored back.
"""

import os

import numpy as np

import concourse.bacc as bacc
import concourse.bass as bass
import concourse.mybir as mybir
import concourse.tile as tile
from concourse.bass_utils import run_bass_kernel_spmd

N = 16_777_216
NCORES = 8
R = N // 4             # rest axis length = 4,194,304
BC = R // NCORES       # per-core elems per (d,t) block = 524,288
P = 128
FC = BC // P           # 4096 free-dim cols per block per core
FT = 2048              # tile width (cols)
NT = FC // FT          # tile iterations

F32 = mybir.dt.float32

_NC = None


def _ensure_ntff_hook():
    """The agent image's antenv lacks axon_hooks; recreate it so that
    run_bass_kernel_spmd(trace=True) can profile via the axon .so."""
    import sys
    import types

    name = "antenv.axon_hooks"
    if name not in sys.modules:
        try:
            import antenv.axon_hooks  # noqa: F401
        except ImportError:
            mod = types.ModuleType(name)
            mod._hook = None
            mod.set_axon_ntff_profile_hook = lambda h: setattr(mod, "_hook", h)
            mod.get_axon_ntff_profile_hook = lambda: mod._hook
            sys.modules[name] = mod
            import antenv

            antenv.axon_hooks = mod
            try:
                from trn_agent_boot.trn_boot import _ntff_profile_via_ctypes

                hook = _ntff_profile_via_ctypes("/opt/axon/libaxon_pjrt.so")
                if hook is not None:
                    mod._hook = hook
            except Exception as e:  # pragma: no cover
                print(f"ntff hook setup failed: {e}", file=sys.stderr)
    # artifact upload needs external storage; skip it when profiling here
    import concourse.bass_utils as bu

    bu.upload_artifacts = lambda tmpdir: "local://" + str(tmpdir)


def _build_nc_raw():
    """Raw bacc kernel (manual semaphores) — avoids Tile's ~7us preamble
    barrier and ~8us epilogue drain/butterfly."""
    from contextlib import ExitStack

    nc = bacc.Bacc(
        "TRN2", target_bir_lowering=False, debug=False, num_devices=NCORES
    )
    xr = nc.declare_dram_parameter("xr", [2, 2, BC], F32, isOutput=False)
    xi = nc.declare_dram_parameter("xi", [2, 2, BC], F32, isOutput=False)
    coef = nc.declare_dram_parameter("coef", [P, 2], F32, isOutput=False)
    out = nc.declare_dram_parameter("out", [2, 2, 2, BC], F32, isOutput=True)

    xr1 = xr[1].rearrange("t (p f) -> p t f", p=P)
    xi1 = xi[1].rearrange("t (p f) -> p t f", p=P)
    or1 = out[0, 1].rearrange("t (p f) -> p t f", p=P)
    oi1 = out[1, 1].rearrange("t (p f) -> p t f", p=P)

    mult = mybir.AluOpType.mult
    add = mybir.AluOpType.add
    sub = mybir.AluOpType.subtract

    with ExitStack() as ctx:
        def sb(name, shape):
            return ctx.enter_context(nc.sbuf_tensor(name, shape, F32))

        coefs = sb("coefs", [P, 2])
        xrt = [sb(f"xrt{i}", [P, 2, FT]) for i in range(NT)]
        xit = [sb(f"xit{i}", [P, 2, FT]) for i in range(NT)]
        orr = [sb(f"orr{i}", [P, 2, FT]) for i in range(NT)]
        oii = [sb(f"oii{i}", [P, 2, FT]) for i in range(NT)]
        # 4 ping-pong tmps shared across iterations (ACT writes, DVE reads)
        tmp = [sb(f"tmp{j}", [P, FT]) for j in range(4)]

        with (
            nc.Block(no_gpsimd_drain=True) as block,
            nc.semaphore("coef_sem") as coef_sem,
            nc.semaphore("cp_sem") as cp_sem,
            nc.semaphore("out_sem") as out_sem,
            nc.semaphore("dve_sem") as dve_sem,
            nc.semaphore("act_sem") as act_sem,
        ):
            in_sems = [nc.alloc_semaphore(f"in_sem{i}") for i in range(NT)]

            @block.sync
            def _(sync):
                for i in range(NT):
                    sl = bass.ts(i, FT)
                    sync.dma_start(out=xrt[i][:], in_=xr1[:, :, sl]).then_inc(
                        in_sems[i], 16
                    )
                    sync.dma_start(out=xit[i][:], in_=xi1[:, :, sl]).then_inc(
                        in_sems[i], 16
                    )
                # d=0 half: pure copy, DRAM->DRAM. Same ring as the loads:
                # ring FIFO means these only drain after the loads, keeping
                # early HBM bandwidth for the rotate pipeline's inputs.
                sync.dma_start(out=out[0, 0], in_=xr[0]).then_inc(cp_sem, 16)
                sync.dma_start(out=out[1, 0], in_=xi[0]).then_inc(cp_sem, 16)
                for i in range(NT):
                    sl = bass.ts(i, FT)
                    # dve_sem counts combines (4/iter): orr[i] done at 4i+3,
                    # oii[i] done at 4i+4
                    sync.wait_ge(dve_sem, 4 * i + 3)
                    sync.dma_start(out=or1[:, :, sl], in_=orr[i][:]).then_inc(
                        out_sem, 16
                    )
                    sync.wait_ge(dve_sem, 4 * i + 4)
                    sync.dma_start(out=oi1[:, :, sl], in_=oii[i][:]).then_inc(
                        out_sem, 16
                    )
                # gate NEFF completion on all writes having landed
                sync.wait_ge(out_sem, 32 * NT)
                sync.wait_ge(cp_sem, 32)

            @block.scalar
            def _(scalar):
                # coef is tiny and needed first; own ring so it completes
                # immediately
                scalar.dma_start(out=coefs[:], in_=coef[:]).then_inc(coef_sem, 16)
                # ACT precomputes the s* products; DVE fuses (x*c) +/- tmp.
                # Mul k (k=0..4*NT-1) targets tmp[k%4]; from k>=4 it must
                # wait for DVE combine k-4 to have consumed that tmp.
                s = coefs[:, 1:2]
                scalar.wait_ge(coef_sem, 16)
                k = 0
                for i in range(NT):
                    scalar.wait_ge(in_sems[i], 32)
                    for t in (0, 1):
                        u = 1 - t
                        for src in (xit, xrt):
                            if k >= 4:
                                scalar.wait_ge(dve_sem, k - 3)
                            scalar.mul(tmp[k % 4][:], src[i][:, u], s).then_inc(
                                act_sem, 1
                            )
                            k += 1

            @block.vector
            def _(vector):
                c = coefs[:, 0:1]
                k = 0
                for i in range(NT):
                    # combine k consumes tmp[k%4] produced by ACT mul k
                    for t in (0, 1):
                        # out_r[1,t] = c*xr[1,t] + s*xi[1,u]
                        vector.wait_ge(act_sem, k + 1)
                        vector.scalar_tensor_tensor(
                            orr[i][:, t], xrt[i][:, t], c, tmp[k % 4][:], mult, add
                        ).then_inc(dve_sem, 1)
                        k += 1
                        # out_i[1,t] = c*xi[1,t] - s*xr[1,u]
                        vector.wait_ge(act_sem, k + 1)
                        vector.scalar_tensor_tensor(
                            oii[i][:, t], xit[i][:, t], c, tmp[k % 4][:], mult, sub
                        ).then_inc(dve_sem, 1)
                        k += 1

    nc.compile()
    _strip_init_barrier(nc)
    return nc


def _strip_init_barrier(nc):
    """Drop the const-AP memsets and the all-engine entry barrier that
    Bass.__init__ emits: nothing in this kernel reads the const APs, and
    the ~6us of Drain+EventSemaphore ping-pong delays the first DMA.
    Patches nc.to_json_bytes to serve the edited BIR."""
    import json as _json

    d = _json.loads(nc.to_json_bytes())
    main = d["functions"][0]["blocks"][0]
    kept, dropped = [], []
    for ins in main["instructions"]:
        if ins.get("opcode") in ("Memset", "Drain", "EventSemaphore"):
            dropped.append(ins.get("name"))
        else:
            kept.append(ins)
    # expect exactly the init pattern: 4 memsets + 5 drains + 6 evsems
    assert len(dropped) == 15, (len(dropped), dropped)
    assert all(
        i.get("opcode") in ("Call", "UnconditionalBranch") for i in kept
    ), [i.get("opcode") for i in kept]
    main["instructions"] = kept
    # exit barrier: engine completion already gated by the final SP waits
    # (out_sem/cp_sem cover every DMA write), so the end-block drains and
    # all-engine EVSEM butterfly are dead weight too
    end = d["functions"][0]["blocks"][-1]
    assert end["name"].endswith("_end"), end["name"]
    end["instructions"] = [
        i
        for i in end["instructions"]
        if i.get("opcode") not in ("Drain", "EventSemaphore")
    ]
    payload = _json.dumps(d).encode()
    nc.to_json_bytes = lambda: payload


def _build_nc_tile():
    nc = bacc.Bacc(
        "TRN2", target_bir_lowering=False, debug=False, num_devices=NCORES
    )
    xr = nc.declare_dram_parameter("xr", [2, 2, BC], F32, isOutput=False)
    xi = nc.declare_dram_parameter("xi", [2, 2, BC], F32, isOutput=False)
    coef = nc.declare_dram_parameter("coef", [P, 2], F32, isOutput=False)
    # out[part(re/im), d, t, r]
    out = nc.declare_dram_parameter("out", [2, 2, 2, BC], F32, isOutput=True)

    with tile.TileContext(nc) as tc:
        with (
            tc.tile_pool(name="const", bufs=1) as const_pool,
            tc.tile_pool(name="inp", bufs=2) as in_pool,
            tc.tile_pool(name="outp", bufs=2) as out_pool,
            tc.tile_pool(name="tmp", bufs=2) as tmp_pool,
        ):
            coefs = const_pool.tile([P, 2], F32)
            nc.sync.dma_start(out=coefs[:], in_=coef[:])
            c = coefs[:, 0:1]
            s = coefs[:, 1:2]

            # ---- d=0 half: pure copy, DRAM->DRAM on the scalar HWDGE ring
            nc.scalar.dma_start(out=out[0, 0], in_=xr[0])
            nc.scalar.dma_start(out=out[1, 0], in_=xi[0])

            # ---- d=1 half: 2x2 rotation over (t, re/im)
            # DRAM views: (p, t, f)
            xr1 = xr[1].rearrange("t (p f) -> p t f", p=P)
            xi1 = xi[1].rearrange("t (p f) -> p t f", p=P)
            or1 = out[0, 1].rearrange("t (p f) -> p t f", p=P)
            oi1 = out[1, 1].rearrange("t (p f) -> p t f", p=P)

            for i in range(NT):
                sl = bass.ts(i, FT)
                xrt = in_pool.tile([P, 2, FT], F32)
                nc.sync.dma_start(out=xrt[:], in_=xr1[:, :, sl])
                xit = in_pool.tile([P, 2, FT], F32)
                nc.sync.dma_start(out=xit[:], in_=xi1[:, :, sl])

                orr = out_pool.tile([P, 2, FT], F32)
                oii = out_pool.tile([P, 2, FT], F32)
                for t in (0, 1):
                    u = 1 - t
                    # scalar-engine (ACT) does the scalar multiplies,
                    # vector-engine (DVE) the adds/subs
                    ta = tmp_pool.tile([P, FT], F32)
                    nc.scalar.mul(ta[:], xrt[:, t], c)       # c*xr[1,t]
                    tb = tmp_pool.tile([P, FT], F32)
                    nc.scalar.mul(tb[:], xit[:, u], s)       # s*xi[1,u]
                    # out_r[1,t] = c*xr[1,t] + s*xi[1,u]
                    nc.vector.tensor_add(orr[:, t], ta[:], tb[:])
                    tg = tmp_pool.tile([P, FT], F32)
                    nc.scalar.mul(tg[:], xit[:, t], c)       # c*xi[1,t]
                    td = tmp_pool.tile([P, FT], F32)
                    nc.scalar.mul(td[:], xrt[:, u], s)       # s*xr[1,u]
                    # out_i[1,t] = c*xi[1,t] - s*xr[1,u]
                    nc.vector.tensor_sub(oii[:, t], tg[:], td[:])

                nc.sync.dma_start(out=or1[:, :, sl], in_=orr[:])
                nc.sync.dma_start(out=oi1[:, :, sl], in_=oii[:])

    nc.compile()
    return nc


def run(x_real, x_imag, angle, trace=False):
    """Returns (full_output (2,N) float32, BassKernelResults)."""
    global _NC
    if _NC is None:
        if os.environ.get("KERNEL_TILE"):
            _NC = _build_nc_tile()
        else:
            _NC = _build_nc_raw()
    if trace:
        _ensure_ntff_hook()

    a = float(np.asarray(angle).reshape(-1)[0])
    c = np.float32(np.cos(0.5 * a))
    s = np.float32(np.sin(0.5 * a))
    coef = np.empty((P, 2), np.float32)
    coef[:, 0] = c
    coef[:, 1] = s

    xr4 = np.asarray(x_real, dtype=np.float32).reshape(2, 2, NCORES, BC)
    xi4 = np.asarray(x_imag, dtype=np.float32).reshape(2, 2, NCORES, BC)
    in_maps = []
    for k in range(NCORES):
        in_maps.append({
            "xr": np.ascontiguousarray(xr4[:, :, k, :]),
            "xi": np.ascontiguousarray(xi4[:, :, k, :]),
            "coef": coef,
        })

    res = run_bass_kernel_spmd(_NC, in_maps, list(range(NCORES)), trace=trace)

    out = np.empty((2, 2, 2, NCORES, BC), np.float32)
    for k in range(NCORES):
        out[:, :, :, k, :] = res.results[k]["out"]
    return out.reshape(2, N), res


def kernel(x_real, x_imag, angle):
    trace = bool(os.environ.get("KERNEL_TRACE"))
    out, _ = run(x_real, x_imag, angle, trace=trace)
    return out


# revision 25
# speedup vs baseline: 1.0090x; 1.0090x over previous
"""Controlled-rotation (CR) kernel for Trainium2, 8-core SPMD.

Math (D=2, control=0, target=1, j=0,k=1  =>  S = Pauli-X):
  x viewed as (control d, target t, rest r) with rest = N/4.
  d=0: identity.  d=1: apply RX(angle): M = cos(a/2) I - i sin(a/2) X.
  With c=cos(a/2), s=sin(a/2), u=x[1,0], v=x[1,1]:
    out_u = (c*ur + s*vi) + i(c*ui - s*vr)
    out_v = (c*vr + s*ui) + i(c*vi - s*ur)
  Output = stack([real, imag]) of shape (2, N), float32.

Sharding: rest axis split evenly across 8 cores (no communication).
Per core: d=0 half is a DRAM->DRAM DMA copy; d=1 half is loaded to SBUF,
rotated with DVE multiply-accumulate ops, stored back.
"""

import os

import numpy as np

import concourse.bacc as bacc
import concourse.bass as bass
import concourse.mybir as mybir
import concourse.tile as tile
from concourse.bass_utils import run_bass_kernel_spmd

N = 16_777_216
NCORES = 8
R = N // 4             # rest axis length = 4,194,304
BC = R // NCORES       # per-core elems per (d,t) block = 524,288
P = 128
FC = BC // P           # 4096 free-dim cols per block per core
FT = 2048              # tile width (cols)
NT = FC // FT          # tile iterations

F32 = mybir.dt.float32

_NC = None


def _ensure_ntff_hook():
    """The agent image's antenv lacks axon_hooks; recreate it so that
    run_bass_kernel_spmd(trace=True) can profile via the axon .so."""
    import sys
    import types

    name = "antenv.axon_hooks"
    if name not in sys.modules:
        try:
            import antenv.axon_hooks  # noqa: F401
        except ImportError:
            mod = types.ModuleType(name)
            mod._hook = None
            mod.set_axon_ntff_profile_hook = lambda h: setattr(mod, "_hook", h)
            mod.get_axon_ntff_profile_hook = lambda: mod._hook
            sys.modules[name] = mod
            import antenv

            antenv.axon_hooks = mod
            try:
                from trn_agent_boot.trn_boot import _ntff_profile_via_ctypes

                hook = _ntff_profile_via_ctypes("/opt/axon/libaxon_pjrt.so")
                if hook is not None:
                    mod._hook = hook
            except Exception as e:  # pragma: no cover
                print(f"ntff hook setup failed: {e}", file=sys.stderr)
    # artifact upload needs external storage; skip it when profiling here
    import concourse.bass_utils as bu

    bu.upload_artifacts = lambda tmpdir: "local://" + str(tmpdir)


def _build_nc_raw():
    """Raw bacc kernel (manual semaphores) — avoids Tile's ~7us preamble
    barrier and ~8us epilogue drain/butterfly."""
    from contextlib import ExitStack

    nc = bacc.Bacc(
        "TRN2", target_bir_lowering=False, debug=False, num_devices=NCORES
    )
    xr = nc.declare_dram_parameter("xr", [2, 2, BC], F32, isOutput=False)
    xi = nc.declare_dram_parameter("xi", [2, 2, BC], F32, isOutput=False)
    coef = nc.declare_dram_parameter("coef", [P, 2], F32, isOutput=False)
    out = nc.declare_dram_parameter("out", [2, 2, 2, BC], F32, isOutput=True)

    xr1 = xr[1].rearrange("t (p f) -> p t f", p=P)
    xi1 = xi[1].rearrange("t (p f) -> p t f", p=P)
    or1 = out[0, 1].rearrange("t (p f) -> p t f", p=P)
    oi1 = out[1, 1].rearrange("t (p f) -> p t f", p=P)

    mult = mybir.AluOpType.mult
    add = mybir.AluOpType.add
    sub = mybir.AluOpType.subtract

    with ExitStack() as ctx:
        def sb(name, shape):
            return ctx.enter_context(nc.sbuf_tensor(name, shape, F32))

        coefs = sb("coefs", [P, 2])
        xrt = [sb(f"xrt{i}", [P, 2, FT]) for i in range(NT)]
        xit = [sb(f"xit{i}", [P, 2, FT]) for i in range(NT)]
        orr = [sb(f"orr{i}", [P, 2, FT]) for i in range(NT)]
        oii = [sb(f"oii{i}", [P, 2, FT]) for i in range(NT)]
        # 4 ping-pong tmps shared across iterations (ACT writes, DVE reads)
        tmp = [sb(f"tmp{j}", [P, FT]) for j in range(4)]

        with (
            nc.Block(no_gpsimd_drain=True) as block,
            nc.semaphore("coef_sem") as coef_sem,
            nc.semaphore("cp_sem") as cp_sem,
            nc.semaphore("out_sem") as out_sem,
            nc.semaphore("dve_sem") as dve_sem,
            nc.semaphore("act_sem") as act_sem,
        ):
            in_sems = [nc.alloc_semaphore(f"in_sem{i}") for i in range(NT)]

            @block.sync
            def _(sync):
                for i in range(NT):
                    sl = bass.ts(i, FT)
                    sync.dma_start(out=xrt[i][:], in_=xr1[:, :, sl]).then_inc(
                        in_sems[i], 16
                    )
                    sync.dma_start(out=xit[i][:], in_=xi1[:, :, sl]).then_inc(
                        in_sems[i], 16
                    )
                # d=0 half: pure copy, DRAM->DRAM. Same ring as the loads:
                # ring FIFO means these only drain after the loads, keeping
                # early HBM bandwidth for the rotate pipeline's inputs.
                sync.dma_start(out=out[0, 0], in_=xr[0]).then_inc(cp_sem, 16)
                sync.dma_start(out=out[1, 0], in_=xi[0]).then_inc(cp_sem, 16)
                for i in range(NT):
                    sl = bass.ts(i, FT)
                    # dve_sem counts combines (4/iter): orr[i] done at 4i+3,
                    # oii[i] done at 4i+4
                    sync.wait_ge(dve_sem, 4 * i + 3)
                    sync.dma_start(out=or1[:, :, sl], in_=orr[i][:]).then_inc(
                        out_sem, 16
                    )
                    sync.wait_ge(dve_sem, 4 * i + 4)
                    sync.dma_start(out=oi1[:, :, sl], in_=oii[i][:]).then_inc(
                        out_sem, 16
                    )
                # gate NEFF completion on all writes having landed
                sync.wait_ge(out_sem, 32 * NT)
                sync.wait_ge(cp_sem, 32)

            @block.scalar
            def _(scalar):
                # coef is tiny and needed first; own ring so it completes
                # immediately
                scalar.dma_start(out=coefs[:], in_=coef[:]).then_inc(coef_sem, 16)
                # ACT precomputes the s* products; DVE fuses (x*c) +/- tmp.
                # Mul k (k=0..4*NT-1) targets tmp[k%4]; from k>=4 it must
                # wait for DVE combine k-4 to have consumed that tmp.
                s = coefs[:, 1:2]
                scalar.wait_ge(coef_sem, 16)
                k = 0
                for i in range(NT):
                    scalar.wait_ge(in_sems[i], 32)
                    for t in (0, 1):
                        u = 1 - t
                        for src in (xit, xrt):
                            if k >= 4:
                                scalar.wait_ge(dve_sem, k - 3)
                            scalar.mul(tmp[k % 4][:], src[i][:, u], s).then_inc(
                                act_sem, 1
                            )
                            k += 1

            @block.vector
            def _(vector):
                c = coefs[:, 0:1]
                k = 0
                for i in range(NT):
                    # combine k consumes tmp[k%4] produced by ACT mul k
                    for t in (0, 1):
                        # out_r[1,t] = c*xr[1,t] + s*xi[1,u]
                        vector.wait_ge(act_sem, k + 1)
                        vector.scalar_tensor_tensor(
                            orr[i][:, t], xrt[i][:, t], c, tmp[k % 4][:], mult, add
                        ).then_inc(dve_sem, 1)
                        k += 1
                        # out_i[1,t] = c*xi[1,t] - s*xr[1,u]
                        vector.wait_ge(act_sem, k + 1)
                        vector.scalar_tensor_tensor(
                            oii[i][:, t], xit[i][:, t], c, tmp[k % 4][:], mult, sub
                        ).then_inc(dve_sem, 1)
                        k += 1

    nc.compile()
    _strip_init_barrier(nc)
    return nc


def _strip_init_barrier(nc):
    """Drop the const-AP memsets and the all-engine entry barrier that
    Bass.__init__ emits: nothing in this kernel reads the const APs, and
    the ~6us of Drain+EventSemaphore ping-pong delays the first DMA.
    Patches nc.to_json_bytes to serve the edited BIR."""
    import json as _json

    d = _json.loads(nc.to_json_bytes())
    main = d["functions"][0]["blocks"][0]
    kept, dropped = [], []
    for ins in main["instructions"]:
        if ins.get("opcode") in ("Memset", "Drain", "EventSemaphore"):
            dropped.append(ins.get("name"))
        else:
            kept.append(ins)
    # expect exactly the init pattern: 4 memsets + 5 drains + 6 evsems
    assert len(dropped) == 15, (len(dropped), dropped)
    assert all(
        i.get("opcode") in ("Call", "UnconditionalBranch") for i in kept
    ), [i.get("opcode") for i in kept]
    main["instructions"] = kept
    # exit barrier: engine completion already gated by the final SP waits
    # (out_sem/cp_sem cover every DMA write), so the end-block drains and
    # all-engine EVSEM butterfly are dead weight too
    end = d["functions"][0]["blocks"][-1]
    assert end["name"].endswith("_end"), end["name"]
    end["instructions"] = [
        i
        for i in end["instructions"]
        if i.get("opcode") not in ("Drain", "EventSemaphore")
    ]
    payload = _json.dumps(d).encode()
    nc.to_json_bytes = lambda: payload


def _build_nc_tile():
    nc = bacc.Bacc(
        "TRN2", target_bir_lowering=False, debug=False, num_devices=NCORES
    )
    xr = nc.declare_dram_parameter("xr", [2, 2, BC], F32, isOutput=False)
    xi = nc.declare_dram_parameter("xi", [2, 2, BC], F32, isOutput=False)
    coef = nc.declare_dram_parameter("coef", [P, 2], F32, isOutput=False)
    # out[part(re/im), d, t, r]
    out = nc.declare_dram_parameter("out", [2, 2, 2, BC], F32, isOutput=True)

    with tile.TileContext(nc) as tc:
        with (
            tc.tile_pool(name="const", bufs=1) as const_pool,
            tc.tile_pool(name="inp", bufs=2) as in_pool,
            tc.tile_pool(name="outp", bufs=2) as out_pool,
            tc.tile_pool(name="tmp", bufs=2) as tmp_pool,
        ):
            coefs = const_pool.tile([P, 2], F32)
            nc.sync.dma_start(out=coefs[:], in_=coef[:])
            c = coefs[:, 0:1]
            s = coefs[:, 1:2]

            # ---- d=0 half: pure copy, DRAM->DRAM on the scalar HWDGE ring
            nc.scalar.dma_start(out=out[0, 0], in_=xr[0])
            nc.scalar.dma_start(out=out[1, 0], in_=xi[0])

            # ---- d=1 half: 2x2 rotation over (t, re/im)
            # DRAM views: (p, t, f)
            xr1 = xr[1].rearrange("t (p f) -> p t f", p=P)
            xi1 = xi[1].rearrange("t (p f) -> p t f", p=P)
            or1 = out[0, 1].rearrange("t (p f) -> p t f", p=P)
            oi1 = out[1, 1].rearrange("t (p f) -> p t f", p=P)

            for i in range(NT):
                sl = bass.ts(i, FT)
                xrt = in_pool.tile([P, 2, FT], F32)
                nc.sync.dma_start(out=xrt[:], in_=xr1[:, :, sl])
                xit = in_pool.tile([P, 2, FT], F32)
                nc.sync.dma_start(out=xit[:], in_=xi1[:, :, sl])

                orr = out_pool.tile([P, 2, FT], F32)
                oii = out_pool.tile([P, 2, FT], F32)
                for t in (0, 1):
                    u = 1 - t
                    # scalar-engine (ACT) does the scalar multiplies,
                    # vector-engine (DVE) the adds/subs
                    ta = tmp_pool.tile([P, FT], F32)
                    nc.scalar.mul(ta[:], xrt[:, t], c)       # c*xr[1,t]
                    tb = tmp_pool.tile([P, FT], F32)
                    nc.scalar.mul(tb[:], xit[:, u], s)       # s*xi[1,u]
                    # out_r[1,t] = c*xr[1,t] + s*xi[1,u]
                    nc.vector.tensor_add(orr[:, t], ta[:], tb[:])
                    tg = tmp_pool.tile([P, FT], F32)
                    nc.scalar.mul(tg[:], xit[:, t], c)       # c*xi[1,t]
                    td = tmp_pool.tile([P, FT], F32)
                    nc.scalar.mul(td[:], xrt[:, u], s)       # s*xr[1,u]
                    # out_i[1,t] = c*xi[1,t] - s*xr[1,u]
                    nc.vector.tensor_sub(oii[:, t], tg[:], td[:])

                nc.sync.dma_start(out=or1[:, :, sl], in_=orr[:])
                nc.sync.dma_start(out=oi1[:, :, sl], in_=oii[:])

    nc.compile()
    return nc


def run(x_real, x_imag, angle, trace=False):
    """Returns (full_output (2,N) float32, BassKernelResults)."""
    global _NC
    if _NC is None:
        if os.environ.get("KERNEL_TILE"):
            _NC = _build_nc_tile()
        else:
            _NC = _build_nc_raw()
    if trace:
        _ensure_ntff_hook()

    a = float(np.asarray(angle).reshape(-1)[0])
    c = np.float32(np.cos(0.5 * a))
    s = np.float32(np.sin(0.5 * a))
    coef = np.empty((P, 2), np.float32)
    coef[:, 0] = c
    coef[:, 1] = s

    xr4 = np.asarray(x_real, dtype=np.float32).reshape(2, 2, NCORES, BC)
    xi4 = np.asarray(x_imag, dtype=np.float32).reshape(2, 2, NCORES, BC)
    in_maps = []
    for k in range(NCORES):
        in_maps.append({
            "xr": np.ascontiguousarray(xr4[:, :, k, :]),
            "xi": np.ascontiguousarray(xi4[:, :, k, :]),
            "coef": coef,
        })

    res = run_bass_kernel_spmd(_NC, in_maps, list(range(NCORES)), trace=trace)

    out = np.empty((2, 2, 2, NCORES, BC), np.float32)
    for k in range(NCORES):
        out[:, :, :, k, :] = res.results[k]["out"]
    return out.reshape(2, N), res


def kernel(x_real, x_imag, angle):
    out, _ = run(x_real, x_imag, angle, trace=False)
    return out
